# revision 57
# baseline (speedup 1.0000x reference)
"""LocallyConnected2D (B=16, 32x32, CIN=COUT=64, 3x3, pad=1) on 8 TRN2 NeuronCores.

Shard the 32 output rows across 8 cores (4 rows each). Weights are repacked on
the host into bf16, per-core, fully contiguous layouts (bf16 halves the HBM
weight stream, the kernel's roofline). Per group of 8 consecutive output
pixels: 5 PSUM-accumulating matmuls with the x patches as the full 128-wide
stationary operand (8 pixels x 16 batch) and N=512 streamed weight columns
(8 pixels x 64 cout). Pixel q's useful products land on the diagonal (rows
[16q,+16) x cols [64q,+64) of the group's PSUM block); off-diagonal products
are garbage that is never consumed. 4 groups per output row accumulate into 4
PSUM banks (psum tile [128, 2048], double buffered = all 8 banks). Compute
engines must access 32-aligned partition starts, so PSUM->SBUF extraction
copies [32, 4, 128] two-pixel blocks (half garbage, split across DVE and ACT),
and the host discards the off-diagonal halves of the bf16 output.

out[b,i,j,o] = sum_{c,k} x_pad[b, i+di, j+dj, c] * W[o, c, i, j, k], k=3*di+dj.

Tap pairs (2p, 2p+1) are stacked on partition halves (64m+cin, m=0/1), with
the relative column shift between the two taps baked into the x tile at DMA
time. Pairs p=0,2,3 all have shift dj0-dj1=-1 and same-row taps, so one tile
per input row serves all three; pair p=1 (rows r,r+1, shift +2) gets its own
tile (both PE operands must share a base partition, so the pair tiles cannot
be replaced by half-tile matmuls). Tap 8 reads the shared tile's top half.
The 10 x tiles live in two packed slabs whose shifts, zero pads, and
partition-half duplication are pre-baked on the host, so each slab loads as
ONE full-width contiguous DMA (no memsets; measured faster than 64-partition
multi-segment loads). Weight DMAs alternate between the SP and ACT HWDGE
rings; output DMAs queue on SP after all loads.

Host layouts (per core c, local row r, i = 4c+r):
  w_pairs [4, 128, 8192] bf16: [r, 64m+cin, g*2048+p*512+q*64+o]
                                 = W[o, cin, i, 8g+q, 2p+m]
  w_sing  [64, 8192]    bf16: [cin, r*2048+g*512+q*64+o] = W[o, cin, i, 8g+q, 8]
  xtall   [128, 3456]   bf16: T(0..5) slabs, xt[s]=x_pad[4c+s] as [cin, j*16+b]
  xtp     [128, 2304]   bf16: P1(0..3) slabs
  out     [4, 128, 512] bf16: [r, 32qq+16hb+b, g*128+hn*64+o]
          valid where hn == hb -> out[b, i, 8g+2qq+hb, o]

x tiles (576 = 36 cols * 16 batch; input col j' stored at (j'+2)*16):
  T(s), s=0..5:  half0 = xt[s] at 32, half1 = xt[s] at 16 (shift -1)
  P1(r), r=0..3: half0 = xt[r] at 32, half1 = xt[r+1] at 64 (shift +2)
  chunk->lhsT for row r, group g (offset (8g+d)*16, width 128):
    p0: T(r)   d=1 | p1: P1(r) d=3 | p2: T(r+1) d=2 | p3: T(r+2) d=1
    tap8: T(r+2)[0:64] d=3
"""

import numpy as np

B, IH, IW, CIN = 16, 32, 32, 64
COUT, OH, OW = 64, 32, 32
NCORES, RPC = 8, 4

_NC = {}


def _build_nc(repeat=1, parts="full", unroll=1):
    import contextlib

    import concourse.bacc as bacc
    import concourse.mybir as mybir
    import concourse.tile as tile

    f32 = mybir.dt.float32
    bf16 = mybir.dt.bfloat16
    nc = bacc.Bacc("TRN2", target_bir_lowering=False, debug=False)
    wp = nc.dram_tensor("w_pairs", [RPC, 128, 8192], bf16, kind="ExternalInput")
    ws = nc.dram_tensor("w_sing", [64, 8192], bf16, kind="ExternalInput")
    xta = nc.dram_tensor("xtall", [128, 6 * 576], bf16, kind="ExternalInput")
    xtp = nc.dram_tensor("xtp", [128, 4 * 576], bf16, kind="ExternalInput")
    out = nc.dram_tensor("out", [RPC, 128, 512], bf16, kind="ExternalOutput")
    wp_ap, ws_ap, out_ap = wp.ap(), ws.ap(), out.ap()
    xta_ap, xtp_ap = xta.ap(), xtp.ap()

    with tile.TileContext(nc) as tc:
        rep = tc.For_i(0, repeat, 1) if repeat > 1 else contextlib.nullcontext()
        with (
            rep,
            tc.tile_pool(name="x", bufs=1) as xpool,
            tc.tile_pool(name="ws", bufs=1) as wspool,
            tc.tile_pool(name="wp", bufs=4) as wppool,
            tc.tile_pool(name="stage", bufs=4) as stage_pool,
            tc.tile_pool(name="psum", bufs=2, space="PSUM") as psum_pool,
        ):
          for _u in range(unroll):
            # persistent x tiles in two packed slabs; shifts, pads, and the
            # partition-half duplication are pre-baked on the host, so each
            # slab is ONE full-width contiguous DMA (no memsets)
            xall = xpool.tile([128, 6 * 576], bf16, tag="xall")
            xp1 = xpool.tile([128, 4 * 576], bf16, tag="xp1")

            # first weight chunk goes out ahead of everything else
            wp_ts = []
            for r in range(RPC):
                wp_row = wppool.tile([128, 8192], bf16, tag="wp", name=f"wp{_u}_{r}")
                wp_ts.append(wp_row)
            ws_t = wspool.tile([64, 8192], bf16, tag="ws")

            nc.sync.dma_start(wp_ts[0][:, 0:2048], wp_ap[0][:, 0:2048])
            nc.sync.dma_start(xall[:], xta_ap)
            nc.scalar.dma_start(xp1[:], xtp_ap)
            nc.scalar.dma_start(ws_t[:, 0:2048], ws_ap[:, 0:2048])
            for g in range(1, 4):
                nc.sync.dma_start(
                    wp_ts[0][:, 2048 * g : 2048 * (g + 1)],
                    wp_ap[0][:, 2048 * g : 2048 * (g + 1)],
                )
            nc.scalar.dma_start(ws_t[:, 2048:8192], ws_ap[:, 2048:8192])
            for r in range(1, RPC):
                nc.sync.dma_start(wp_ts[r][:, 0:4096], wp_ap[r][:, 0:4096])
                nc.scalar.dma_start(wp_ts[r][:, 4096:8192], wp_ap[r][:, 4096:8192])

            T = [xall[:, 576 * s : 576 * (s + 1)] for s in range(6)]
            P1 = [xp1[:, 576 * s : 576 * (s + 1)] for s in range(RPC)]

            copy_ops = [
                nc.vector.tensor_copy, nc.vector.tensor_copy,
                nc.scalar.copy, nc.scalar.copy,
            ]
            for r in range(RPC):
                wp_t = wp_ts[r]

                # (x tile, partition range, stationary col offset d,
                #  weight chunk index, weight partition range)
                chunks = [
                    (T[r], 0, 128, 1, 0, 0, 128),
                    (P1[r], 0, 128, 3, 1, 0, 128),
                    (T[r + 1], 0, 128, 2, 2, 0, 128),
                    (T[r + 2], 0, 128, 1, 3, 0, 128),
                    (T[r + 2], 0, 64, 3, 4, 0, 64),
                ]
                if parts == "dma":
                    continue
                ps = psum_pool.tile([128, 2048], f32, tag="ps")
                for g in range(4):
                    for ci, (xtile, k0, k1, d, wc, wk0, wk1) in enumerate(chunks):
                        lo = (8 * g + d) * 16
                        lhsT = xtile[k0:k1, lo : lo + 128]
                        if wc < 4:
                            rhs = wp_t[
                                wk0:wk1,
                                2048 * g + 512 * wc : 2048 * g + 512 * (wc + 1),
                            ]
                        else:
                            rhs = ws_t[:, 2048 * r + 512 * g : 2048 * r + 512 * (g + 1)]
                        nc.tensor.matmul(
                            ps[:, 512 * g : 512 * (g + 1)],
                            lhsT,
                            rhs,
                            start=(ci == 0),
                            stop=(ci == len(chunks) - 1),
                        )
                if parts == "mm":
                    continue
                # pixel q's outputs sit at psum rows [16q,+16), cols
                # [64q,+64) of each group block (diagonal). Engine accesses
                # must start at 32-aligned partitions, so copy [32, 128]
                # two-pixel blocks (half garbage); host discards the
                # off-diagonal halves.
                stage = stage_pool.tile([128, 512], bf16, tag="stage")
                for qq in range(4):
                    src = ps[32 * qq : 32 * qq + 32, :].rearrange(
                        "p (g n) -> p g n", g=4
                    )[:, :, 128 * qq : 128 * qq + 128]
                    copy_ops[qq](
                        stage[32 * qq : 32 * qq + 32, :].rearrange(
                            "p (g n) -> p g n", g=4
                        ),
                        src,
                    )
                nc.sync.dma_start(out_ap[r], stage[:])
    nc.compile()
    return nc


def _repack_inputs(x, weight):
    import ml_dtypes

    bf16 = ml_dtypes.bfloat16
    x = np.asarray(x, dtype=np.float32)
    weight = np.asarray(weight, dtype=np.float32)

    # weight: [o, c, i, j, k] -> cast once (contiguous pass), then bf16
    # strided gather passes
    wbf = weight.astype(bf16)
    w8 = wbf[..., :8].reshape(COUT, CIN, OH, 4, 8, 4, 2)  # [o,c,i,g,q,p,m]
    wp = np.ascontiguousarray(w8.transpose(2, 6, 1, 3, 5, 4, 0)).reshape(
        OH, 128, 8192
    )  # [i, 64m+c, (g p q o)]
    s = wbf[..., 8].reshape(COUT, CIN, OH, 4, 8)  # [o, c, i, g, q]
    ws = np.ascontiguousarray(s.transpose(1, 2, 3, 4, 0))  # [c,i,g,q,o]

    xpad = np.zeros((IH + 2, CIN, IW, B), dtype=bf16)
    xpad[1:33] = x.transpose(1, 3, 2, 0)  # [ih, c, j, b]

    in_maps = []
    for c in range(NCORES):
        xt6 = xpad[c * RPC : c * RPC + RPC + 2].reshape(RPC + 2, CIN, 512)
        xt6t = xt6.transpose(1, 0, 2)  # [c, s, 512]
        xtall = np.zeros((128, RPC + 2, 576), dtype=bf16)
        xtall[0:64, :, 32:544] = xt6t
        xtall[64:128, :, 16:528] = xt6t
        xtp = np.zeros((128, RPC, 576), dtype=bf16)
        xtp[0:64, :, 32:544] = xt6t[:, 0:RPC]
        xtp[64:128, :, 64:576] = xt6t[:, 1 : RPC + 1]
        in_maps.append(
            {
                "w_pairs": wp[c * RPC : (c + 1) * RPC],
                "w_sing": np.ascontiguousarray(
                    ws[:, c * RPC : (c + 1) * RPC]
                ).reshape(64, 8192),
                "xtall": xtall.reshape(128, (RPC + 2) * 576),
                "xtp": xtp.reshape(128, RPC * 576),
            }
        )
    return in_maps


def _get_nc(repeat=1, parts="full", unroll=1):
    key = (repeat, parts, unroll)
    if key not in _NC:
        _NC[key] = _build_nc(repeat, parts, unroll)
    return _NC[key]


def run_spmd(in_maps, **kwargs):
    from concourse.bass_utils import run_bass_kernel_spmd

    return run_bass_kernel_spmd(
        _get_nc(), in_maps, core_ids=list(range(NCORES)), **kwargs
    )


_RUNNER = None


def _get_runner():
    """Build the sharded PJRT executable once; later kernel() calls only pay
    input transfer + execution (no jit retrace / recompile)."""
    global _RUNNER
    if _RUNNER is not None:
        return _RUNNER

    import jax
    import jax.numpy as jnp
    from jax.experimental.shard_map import shard_map
    from jax.sharding import Mesh, NamedSharding, PartitionSpec

    import concourse.mybir as mybir
    from concourse import bass2jax

    bass2jax.install_neuronx_cc_hook()
    nc = _get_nc()

    partition_name = nc.partition_id_tensor.name if nc.partition_id_tensor else None
    in_names, out_names, out_avals = [], [], []
    for alloc in nc.m.functions[0].allocations:
        if not isinstance(alloc, mybir.MemoryLocationSet):
            continue
        name = alloc.memorylocations[0].name
        if alloc.kind == "ExternalInput":
            if name != partition_name:
                in_names.append(name)
        elif alloc.kind == "ExternalOutput":
            out_names.append(name)
            out_avals.append(
                jax.core.ShapedArray(
                    tuple(alloc.tensor_shape), mybir.dt.np(alloc.dtype)
                )
            )
    n_params = len(in_names)
    n_outs = len(out_avals)
    all_in_names = list(in_names) + list(out_names)
    if partition_name is not None:
        all_in_names.append(partition_name)

    def _body(*args):
        operands = list(args)
        if partition_name is not None:
            operands.append(bass2jax.partition_id_tensor())
        return tuple(
            bass2jax._bass_exec_p.bind(
                *operands,
                out_avals=tuple(out_avals),
                in_names=tuple(all_in_names),
                out_names=tuple(out_names),
                lowering_input_output_aliases=(),
                sim_require_finite=True,
                sim_require_nnan=True,
                nc=nc,
            )
        )

    devices = jax.devices()[:NCORES]
    mesh = Mesh(np.asarray(devices), ("core",))
    spec = NamedSharding(mesh, PartitionSpec("core"))
    sharded = jax.jit(
        shard_map(
            _body,
            mesh=mesh,
            in_specs=(PartitionSpec("core"),) * (n_params + n_outs),
            out_specs=(PartitionSpec("core"),) * n_outs,
            check_rep=False,
        ),
        donate_argnums=tuple(range(n_params, n_params + n_outs)),
        keep_unused=True,
    )
    zshapes = [(NCORES * a.shape[0], *a.shape[1:]) for a in out_avals]
    zdtypes = [a.dtype for a in out_avals]
    zfn = jax.jit(
        lambda: tuple(jnp.zeros(s, d) for s, d in zip(zshapes, zdtypes)),
        out_shardings=tuple(spec for _ in zshapes),
    )

    def run(in_maps):
        concat = [
            np.concatenate([np.asarray(in_maps[c][nm]) for c in range(NCORES)])
            for nm in in_names
        ]
        dev_in = [jax.device_put(a, spec) for a in concat]
        outs = sharded(*dev_in, *zfn())
        results = []
        for c in range(NCORES):
            m = {}
            for i, nm in enumerate(out_names):
                m[nm] = np.asarray(outs[i]).reshape(
                    NCORES, *out_avals[i].shape
                )[c]
            results.append(m)
        return results

    _RUNNER = run
    return run


def kernel(x, weight, bias, _results=None):
    if _results is None:
        in_maps = _repack_inputs(x, weight)
        try:
            _results = _get_runner()(in_maps)
        except Exception:
            _results = run_spmd(in_maps).results
    arr = np.stack([np.asarray(r["out"], dtype=np.float32) for r in _results])
    a = arr.reshape(NCORES, RPC, 4, 2, 16, 4, 2, 64)  # [c,r,qq,hb,b,g,hn,o]
    good = np.stack([a[:, :, :, 0, :, :, 0, :], a[:, :, :, 1, :, :, 1, :]], axis=3)
    # good: [c, r, qq, hb, b, g, o]; j = 8g + 2qq + hb
    out = good.transpose(4, 0, 1, 5, 2, 3, 6).reshape(B, OH, OW, COUT)
    return out + np.asarray(bias, dtype=np.float32)[None]
